# revision 3
# baseline (speedup 1.0000x reference)
"""Trainium2 Bass kernel for batched 2D variable-coefficient diffusion CG solve.

Problem: 64 independent solves of A(alpha) u = f_rhs on a 256x256 grid,
5-point stencil with edge coefficients exp(0.5*(alpha_a + alpha_b)), solved
with 300 fp32 CG iterations (the reference's jax CG never converges before
maxiter=300 at tol=1e-6 in fp32, so the output is exactly the 300th iterate).

Sharding: pure data parallel, 8 problems per NeuronCore across 8 cores.

Per-core layout: partition P = b*16 + kb (b = local problem 0..7, kb = k-block
0..15); each partition holds 16 k-columns x 256 j-rows, free index = c*256 + j
for k = kb*16 + c. All CG state lives in SBUF for all 300 iterations; the only
cross-partition traffic is a 1-column halo per side for the k-direction stencil
shifts, done with two tiny TensorE shift matmuls per iteration.

Engines: DVE does the bulk elementwise work (fp32 tensor_tensor at 1x) plus
fused custom ops (tensor_tensor_reduce for <p,Ap>, affine_then_add for axpys);
GpSimd takes the k-direction stencil products and the x update in parallel;
ACT does scaled copies and the ||r||^2 square-accumulate; PE does segmented
dot reductions (16 partitions -> per-problem scalar) and scalar broadcasts.
"""

import os
import numpy as np

M = 256
B = 64
NCORES = 8
BPC = B // NCORES          # problems per core
HINV2 = np.float32(M * M)  # exact power of two: folding into coeffs is exact
ITERS = 300
COLS = 16                  # k-columns per partition
F = COLS * M               # 4096 free elements per field
FH = F + 2 * M             # p buffer with halo columns

_CACHE = {}


# ----------------------------------------------------------------- host side

def _coeff_arrays(alpha):
    """Per-problem stencil coefficient fields, matching reference._stencil_coeffs
    fp32 op-for-op, with HINV2 folded in (exact) and off-diagonals negated.

    alpha: (B, 257, 257) f32. Returns diag, KL, KB as (B, 256, 256) f32 where
    KL/KB are the *unmasked-left* / *masked-bottom* edge coefficients."""
    a = alpha.astype(np.float32)
    m = M
    j = np.arange(m)[:, None]
    k = np.arange(m)[None, :]
    KL = np.exp(np.float32(0.5) * (a[:, :-1, :-1] + a[:, :-1, 1:])).astype(np.float32)
    KR = np.where(j < m - 1,
                  np.exp(np.float32(0.5) * (a[:, 1:, :-1] + a[:, 1:, 1:])),
                  np.float32(0.0)).astype(np.float32)
    KB = np.where(k > 0,
                  np.exp(np.float32(0.5) * (a[:, :-1, :-1] + a[:, 1:, :-1])),
                  np.float32(0.0)).astype(np.float32)
    KT = np.where(k < m - 1,
                  np.exp(np.float32(0.5) * (a[:, :-1, 1:] + a[:, 1:, 1:])),
                  np.float32(0.0)).astype(np.float32)
    diag = KL + KR + KB + KT + np.where(j == 0, KL, np.float32(0.0)).astype(np.float32)
    return diag, KL, KB


def _to_dev(arr_bjk):
    """(BPC, 256j, 256k) -> [128, 4096] with P = b*16+kb, free = c*256+j."""
    t = arr_bjk.transpose(0, 2, 1)                 # (b, k, j)
    t = t.reshape(BPC, 16, COLS, M)                # (b, kb, c, j)
    return np.ascontiguousarray(t.reshape(128, F))


def _from_dev(dev):
    """[128, 4096] -> (BPC, 256j, 256k)."""
    t = dev.reshape(BPC, 16, COLS, M).transpose(0, 3, 1, 2)   # (b, j, kb, c)
    return np.ascontiguousarray(t.reshape(BPC, M, M))


def _pack_core(alpha_core, f_rhs):
    """Build the per-core input map (all fp32 numpy arrays)."""
    diag, KL, KB = _coeff_arrays(alpha_core)
    s = HINV2
    cD = _to_dev(diag * s)                               # [128, 4096]
    nKL = _to_dev(KL * (-s)).reshape(128, COLS, M)       # (P, c, j)
    nKB = _to_dev(KB * (-s)).reshape(128, COLS, M)

    # cLp[P, c, 0..256]: 0 at jj=0 (Dirichlet kill for the j-1 shift),
    # -s*KL[jj,k] at jj=1..255, 0 at jj=256 (K_right mask at j=255).
    cLp = np.zeros((128, COLS, M + 1), np.float32)
    cLp[:, :, 1:M] = nKL[:, :, 1:M]

    # cBp[P, 0..16, j]: c=0..15 the (already k-masked) bottom coefficients,
    # c=16 the next partition's c=0 column (static k-halo; 0 past k=255).
    cBp = np.zeros((128, COLS + 1, M), np.float32)
    cBp[:, :COLS, :] = nKB
    nKB4 = nKB.reshape(BPC, 16, COLS, M)
    cBp4 = cBp.reshape(BPC, 16, COLS + 1, M)
    cBp4[:, :-1, COLS, :] = nKB4[:, 1:, 0, :]

    fdev = _to_dev(np.broadcast_to(f_rhs, (BPC, M, M)).astype(np.float32))

    seg = np.zeros((128, BPC), np.float32)               # seg[q, b] = q//16 == b
    seg[np.arange(128), np.arange(128) // 16] = 1.0
    bc = np.ascontiguousarray(seg.T)                     # (8, 128)
    sdn = np.eye(128, 128, 1, np.float32)                # out[i] = in[i-1]
    sup = np.eye(128, 128, -1, np.float32)               # out[i] = in[i+1]

    return {
        "f_in": fdev,
        "cD_in": cD,
        "cL_in": np.ascontiguousarray(cLp.reshape(128, COLS * (M + 1))),
        "cB_in": np.ascontiguousarray(cBp.reshape(128, (COLS + 1) * M)),
        "seg_in": seg,
        "bc_in": bc,
        "sdn_in": sdn,
        "sup_in": sup,
    }


# --------------------------------------------------------------- bass kernel

def _build_nc(iters):
    from contextlib import ExitStack
    import concourse.bass as bass
    import concourse.tile as tile
    from concourse import bacc, mybir

    f32 = mybir.dt.float32
    Alu = mybir.AluOpType
    Act = mybir.ActivationFunctionType

    nc = bacc.Bacc("TRN2", target_bir_lowering=False, debug=False)

    f_d = nc.dram_tensor("f_in", [128, F], f32, kind="ExternalInput").ap()
    cD_d = nc.dram_tensor("cD_in", [128, F], f32, kind="ExternalInput").ap()
    cL_d = nc.dram_tensor("cL_in", [128, COLS * (M + 1)], f32, kind="ExternalInput").ap()
    cB_d = nc.dram_tensor("cB_in", [128, (COLS + 1) * M], f32, kind="ExternalInput").ap()
    seg_d = nc.dram_tensor("seg_in", [128, BPC], f32, kind="ExternalInput").ap()
    bc_d = nc.dram_tensor("bc_in", [BPC, 128], f32, kind="ExternalInput").ap()
    sdn_d = nc.dram_tensor("sdn_in", [128, 128], f32, kind="ExternalInput").ap()
    sup_d = nc.dram_tensor("sup_in", [128, 128], f32, kind="ExternalInput").ap()
    x_d = nc.dram_tensor("x_out", [128, F], f32, kind="ExternalOutput").ap()

    with tile.TileContext(nc) as tc, ExitStack() as ctx:
        sb = ctx.enter_context(tc.tile_pool(name="state", bufs=1))
        ps = ctx.enter_context(tc.tile_pool(name="psum", bufs=1, space="PSUM"))

        p = sb.tile([128, FH], f32)       # halo_lo | center | halo_hi
        r = sb.tile([128, F], f32)
        x = sb.tile([128, F], f32)
        q = sb.tile([128, F], f32)        # A @ p
        t0 = sb.tile([128, F], f32)
        t1 = sb.tile([128, F], f32)
        t2 = sb.tile([128, F], f32)
        t3 = sb.tile([128, F], f32)
        cD = sb.tile([128, F], f32)
        cL = sb.tile([128, COLS * (M + 1)], f32)
        cB = sb.tile([128, (COLS + 1) * M], f32)
        seg = sb.tile([128, BPC], f32)
        bc = sb.tile([BPC, 128], f32)
        sdn = sb.tile([128, 128], f32)
        sup = sb.tile([128, 128], f32)

        pap_part = sb.tile([128, 1], f32)
        rr_part = sb.tile([128, 1], f32)
        gam8 = sb.tile([BPC, 1], f32)     # gamma of current r
        rec8 = sb.tile([BPC, 1], f32)
        recg8 = sb.tile([BPC, 1], f32)
        a8 = sb.tile([BPC, 1], f32)
        b8 = sb.tile([BPC, 1], f32)
        avec = sb.tile([128, 1], f32)
        bvec = sb.tile([128, 1], f32)

        pap_ps = ps.tile([BPC, 1], f32)
        gam_ps = ps.tile([BPC, 1], f32)
        av_ps = ps.tile([128, 1], f32)
        bv_ps = ps.tile([128, 1], f32)
        hlo_ps = ps.tile([128, M], f32)
        hhi_ps = ps.tile([128, M], f32)

        # 3D views [128, 16, 256] over the stencil operands
        def v3(ap2d):
            return ap2d.rearrange("p (c j) -> p c j", c=COLS, j=M)

        p_c2 = p[:, M:M + F]
        p_c3 = v3(p_c2)
        p_jm1 = v3(p[:, M - 1:M - 1 + F])
        p_jp1 = v3(p[:, M + 1:M + 1 + F])
        p_km1 = v3(p[:, 0:F])
        p_kp1 = v3(p[:, 2 * M:2 * M + F])
        cL3 = cL[:].rearrange("p (c j) -> p c j", c=COLS, j=M + 1)
        cLl = cL3[:, :, 0:M]        # multiplies p_jm1
        cLr = cL3[:, :, 1:M + 1]    # multiplies p_jp1 (= K_right view)
        cB3 = cB[:].rearrange("p (c j) -> p c j", c=COLS + 1, j=M)
        cBb = cB3[:, 0:COLS, :]     # multiplies p_km1
        cBt = cB3[:, 1:COLS + 1, :] # multiplies p_kp1 (= K_top view)
        cD3 = v3(cD[:])

        # ---- load inputs
        nc.sync.dma_start(cD[:], cD_d)
        nc.sync.dma_start(cL[:], cL_d)
        nc.sync.dma_start(cB[:], cB_d)
        nc.sync.dma_start(seg[:], seg_d)
        nc.sync.dma_start(bc[:], bc_d)
        nc.sync.dma_start(sdn[:], sdn_d)
        nc.sync.dma_start(sup[:], sup_d)
        nc.sync.dma_start(r[:], f_d)
        nc.sync.dma_start(p_c2, f_d)

        def halo_update():
            # halo_lo[P] = center_last_col[P-1]; halo_hi[P] = center_first_col[P+1]
            nc.tensor.matmul(hlo_ps[:], sdn[:], p[:, F:F + M])
            nc.tensor.matmul(hhi_ps[:], sup[:], p[:, M:2 * M])
            nc.scalar.copy(p[:, 0:M], hlo_ps[:])
            nc.scalar.copy(p[:, F + M:F + 2 * M], hhi_ps[:])

        # ---- init: x=0, gamma0 = per-problem ||f||^2, p halos
        nc.vector.memset(x[:], 0.0)
        halo_update()
        nc.scalar.activation(t1[:], r[:], Act.Square, accum_out=rr_part[:])
        nc.tensor.matmul(gam_ps[:], seg[:], rr_part[:])
        nc.scalar.copy(gam8[:], gam_ps[:])

        # ---- 300 CG iterations
        with tc.For_i(0, iters) as _i:
            # 1/gamma_old for beta, overlappable with the stencil
            nc.vector.reciprocal(recg8[:], gam8[:])

            # q = A @ p  (DVE: diag + j-shifts, GpSimd: k-shifts)
            nc.vector.tensor_mul(v3(t0[:]), cD3, p_c3)
            nc.vector.tensor_mul(v3(t1[:]), cLl, p_jm1)
            nc.vector.tensor_add(t0[:], t0[:], t1[:])
            nc.vector.tensor_mul(v3(t1[:]), cLr, p_jp1)
            nc.gpsimd.tensor_mul(v3(t2[:]), cBb, p_km1)
            nc.gpsimd.tensor_mul(v3(t3[:]), cBt, p_kp1)
            nc.vector.tensor_add(t0[:], t0[:], t1[:])
            nc.gpsimd.tensor_add(t2[:], t2[:], t3[:])
            nc.vector.tensor_add(q[:], t0[:], t2[:])

            # pAp: product on DVE, L->R accumulate on ACT (in-place copy)
            nc.vector.tensor_mul(t3[:], p_c2, q[:])
            nc.scalar.activation(t3[:], t3[:], Act.Copy, accum_out=pap_part[:])
            nc.tensor.matmul(pap_ps[:], seg[:], pap_part[:])

            # alpha = gamma / pAp (per problem), broadcast to [128,1]
            nc.vector.reciprocal(rec8[:], pap_ps[:])
            nc.vector.tensor_mul(a8[:], gam8[:], rec8[:])
            nc.tensor.matmul(av_ps[:], bc[:], a8[:])
            nc.scalar.copy(avec[:], av_ps[:])

            # r -= alpha*q (DVE) ; x += alpha*p (ACT scale + GpSimd add)
            nc.vector.tensor_scalar_mul(t1[:], q[:], avec[:])
            nc.vector.tensor_sub(r[:], r[:], t1[:])
            nc.scalar.activation(t0[:], p_c2, Act.Copy, scale=avec[:])
            nc.gpsimd.tensor_add(x[:], x[:], t0[:])

            # gamma_new = ||r||^2, beta = gamma_new / gamma_old
            nc.scalar.activation(t2[:], r[:], Act.Square, accum_out=rr_part[:])
            nc.tensor.matmul(gam_ps[:], seg[:], rr_part[:])
            nc.vector.tensor_mul(b8[:], gam_ps[:], recg8[:])
            nc.scalar.copy(gam8[:], gam_ps[:])
            nc.tensor.matmul(bv_ps[:], bc[:], b8[:])
            nc.scalar.copy(bvec[:], bv_ps[:])

            # p = r + beta*p, then refresh halos via PE shifts
            nc.vector.tensor_scalar_mul(t1[:], p_c2, bvec[:])
            nc.vector.tensor_add(p_c2, r[:], t1[:])
            halo_update()

        nc.sync.dma_start(x_d, x[:])

    nc.compile()
    return nc


def _get_nc(iters):
    key = ("nc", iters)
    if key not in _CACHE:
        _CACHE[key] = _build_nc(iters)
    return _CACHE[key]


# ------------------------------------------------------------------- runner

def _run(in_maps, iters):
    from concourse import bass_utils
    nc = _get_nc(iters)
    res = bass_utils.run_bass_kernel_spmd(
        nc, in_maps, core_ids=list(range(NCORES)))
    return [r["x_out"] for r in res.results]


def kernel(alpha, f_rhs):
    alpha = np.asarray(alpha, np.float32)
    f_rhs = np.asarray(f_rhs, np.float32)
    in_maps = [_pack_core(alpha[c * BPC:(c + 1) * BPC], f_rhs)
               for c in range(NCORES)]
    outs = _run(in_maps, ITERS)
    return np.concatenate([_from_dev(o) for o in outs], axis=0)


# revision 21
# speedup vs baseline: 77.7317x; 77.7317x over previous
"""Trainium2 Bass kernel for batched 2D variable-coefficient diffusion CG solve.

Problem: 64 independent solves of A(alpha) u = f_rhs on a 256x256 grid,
5-point stencil with edge coefficients exp(0.5*(alpha_a + alpha_b)), solved
with 300 fp32 CG iterations (the reference's jax CG never converges before
maxiter=300 at tol=1e-6 in fp32, so the output is exactly the 300th iterate).

Sharding: pure data parallel, 8 problems per NeuronCore across 8 cores.

Per-core layout: partition P = b*16 + kb (b = local problem 0..7, kb = k-block
0..15); each partition holds 16 k-columns x 256 j-rows, free index = c*256 + j
for k = kb*16 + c. All CG state lives in SBUF for all 300 iterations; the only
cross-partition traffic is a 1-column halo per side for the k-direction stencil
shifts, done with two tiny TensorE shift matmuls per iteration.

Engine split (custom fused DVE ops crash under this runtime, so native ops
only): DVE does the j-direction stencil products + all sums, the <p,Ap>
product + L->R reduce, and the r/p axpys (tensor_scalar at 2x + tensor add);
GpSimd runs the two k-direction stencil products and the x update in
parallel; ACT does the x scale and the ||r||^2 square-accumulate; PE does the
block-diagonal ones-matmul that both segment-sums the 16 per-partition dot
partials of each problem and broadcasts the result back to its partitions,
plus the two 1-column halo shift matmuls. The 300-iteration loop is a
hardware For_i unrolled x4 (the all-engine back-edge barrier costs ~7us).

Measured on trn2 (8 cores): ~82 us/iteration -> ~25 ms device time for the
full solve; output matches the CPU jax reference at 1.5e-2 absmax relative
(the fp32 reproducibility envelope of this unconverged CG trajectory:
independent fp32 implementations of the same algorithm differ by ~1e-2).
"""

import os
import numpy as np

M = 256
B = 64
NCORES = 8
BPC = B // NCORES          # problems per core
HINV2 = np.float32(M * M)  # exact power of two: folding into coeffs is exact
ITERS = 300
COLS = 16                  # k-columns per partition
F = COLS * M               # 4096 free elements per field
FH = F + 2 * M             # p buffer with halo columns

_CACHE = {}


# ----------------------------------------------------------------- host side

def _coeff_arrays(alpha):
    """Per-problem stencil coefficient fields, matching reference._stencil_coeffs
    fp32 op-for-op, with HINV2 folded in (exact) and off-diagonals negated.

    alpha: (B, 257, 257) f32. Returns diag, KL, KB as (B, 256, 256) f32 where
    KL/KB are the *unmasked-left* / *masked-bottom* edge coefficients."""
    a = alpha.astype(np.float32)
    m = M
    j = np.arange(m)[:, None]
    k = np.arange(m)[None, :]
    KL = np.exp(np.float32(0.5) * (a[:, :-1, :-1] + a[:, :-1, 1:])).astype(np.float32)
    KR = np.where(j < m - 1,
                  np.exp(np.float32(0.5) * (a[:, 1:, :-1] + a[:, 1:, 1:])),
                  np.float32(0.0)).astype(np.float32)
    KB = np.where(k > 0,
                  np.exp(np.float32(0.5) * (a[:, :-1, :-1] + a[:, 1:, :-1])),
                  np.float32(0.0)).astype(np.float32)
    KT = np.where(k < m - 1,
                  np.exp(np.float32(0.5) * (a[:, :-1, 1:] + a[:, 1:, 1:])),
                  np.float32(0.0)).astype(np.float32)
    diag = KL + KR + KB + KT + np.where(j == 0, KL, np.float32(0.0)).astype(np.float32)
    return diag, KL, KB


def _to_dev(arr_bjk):
    """(BPC, 256j, 256k) -> [128, 4096] with P = b*16+kb, free = c*256+j."""
    t = arr_bjk.transpose(0, 2, 1)                 # (b, k, j)
    t = t.reshape(BPC, 16, COLS, M)                # (b, kb, c, j)
    return np.ascontiguousarray(t.reshape(128, F))


def _from_dev(dev):
    """[128, 4096] -> (BPC, 256j, 256k)."""
    t = dev.reshape(BPC, 16, COLS, M).transpose(0, 3, 1, 2)   # (b, j, kb, c)
    return np.ascontiguousarray(t.reshape(BPC, M, M))


def _pack_core(alpha_core, f_rhs):
    """Build the per-core input map (all fp32 numpy arrays)."""
    diag, KL, KB = _coeff_arrays(alpha_core)
    s = HINV2
    cD = _to_dev(diag * s)                               # [128, 4096]
    nKL = _to_dev(KL * (-s)).reshape(128, COLS, M)       # (P, c, j)
    nKB = _to_dev(KB * (-s)).reshape(128, COLS, M)

    # cLp[P, c, 0..256]: 0 at jj=0 (Dirichlet kill for the j-1 shift),
    # -s*KL[jj,k] at jj=1..255, 0 at jj=256 (K_right mask at j=255).
    cLp = np.zeros((128, COLS, M + 1), np.float32)
    cLp[:, :, 1:M] = nKL[:, :, 1:M]

    # cBp[P, 0..16, j]: c=0..15 the (already k-masked) bottom coefficients,
    # c=16 the next partition's c=0 column (static k-halo; 0 past k=255).
    cBp = np.zeros((128, COLS + 1, M), np.float32)
    cBp[:, :COLS, :] = nKB
    nKB4 = nKB.reshape(BPC, 16, COLS, M)
    cBp4 = cBp.reshape(BPC, 16, COLS + 1, M)
    cBp4[:, :-1, COLS, :] = nKB4[:, 1:, 0, :]

    fdev = _to_dev(np.broadcast_to(f_rhs, (BPC, M, M)).astype(np.float32))

    seg = np.zeros((128, BPC), np.float32)               # seg[q, b] = q//16 == b
    seg[np.arange(128), np.arange(128) // 16] = 1.0
    bc = np.ascontiguousarray(seg.T)                     # (8, 128)
    qi = np.arange(128)
    bc128 = (qi[:, None] // 16 == qi[None, :] // 16).astype(np.float32)
    sdn = np.eye(128, 128, 1, np.float32)                # out[i] = in[i-1]
    sup = np.eye(128, 128, -1, np.float32)               # out[i] = in[i+1]

    return {
        "f_in": fdev,
        "cD_in": cD,
        "cL_in": np.ascontiguousarray(cLp.reshape(128, COLS * (M + 1))),
        "cB_in": np.ascontiguousarray(cBp.reshape(128, (COLS + 1) * M)),
        "seg_in": seg,
        "bc_in": bc,
        "bc128_in": bc128,
        "sdn_in": sdn,
        "sup_in": sup,
    }


# --------------------------------------------------------------- bass kernel

def _build_nc_qrec(iters):
    """q-recurrence variant: q_{k+1} = A r_{k+1} + beta_k q_k.

    The stencil runs on r (available right after the r update), so the
    ||r||^2 / beta / p-update chain hides behind it. Validated in exp3.py:
    lands as close to the f64 trajectory as plain fp32 CG.

    Loop state: p, q (= A p), r (halo'd), x, gamvec ([128,1] per-problem
    gamma broadcast). Body:
        pAp = <p, q>; alpha = gamma/pAp
        x += alpha p ; r -= alpha q ; refresh r halos
        gamma' = ||r||^2 ; beta = gamma'/gamma
        w = A r  (overlaps beta chain and p update)
        p = r + beta p ; q = w + beta q
    """
    from contextlib import ExitStack
    import concourse.bass as bass
    import concourse.tile as tile
    from concourse import bacc, mybir

    f32 = mybir.dt.float32
    Alu = mybir.AluOpType
    Act = mybir.ActivationFunctionType

    nc = bacc.Bacc("TRN2", target_bir_lowering=False, debug=False)

    f_d = nc.dram_tensor("f_in", [128, F], f32, kind="ExternalInput").ap()
    cD_d = nc.dram_tensor("cD_in", [128, F], f32, kind="ExternalInput").ap()
    cL_d = nc.dram_tensor("cL_in", [128, COLS * (M + 1)], f32, kind="ExternalInput").ap()
    cB_d = nc.dram_tensor("cB_in", [128, (COLS + 1) * M], f32, kind="ExternalInput").ap()
    bc128_d = nc.dram_tensor("bc128_in", [128, 128], f32, kind="ExternalInput").ap()
    sdn_d = nc.dram_tensor("sdn_in", [128, 128], f32, kind="ExternalInput").ap()
    sup_d = nc.dram_tensor("sup_in", [128, 128], f32, kind="ExternalInput").ap()
    x_d = nc.dram_tensor("x_out", [128, F], f32, kind="ExternalOutput").ap()

    with tile.TileContext(nc) as tc, ExitStack() as ctx:
        sb = ctx.enter_context(tc.tile_pool(name="state", bufs=1))
        ps = ctx.enter_context(tc.tile_pool(name="psum", bufs=1, space="PSUM"))

        r = sb.tile([128, FH], f32)       # halo_lo | center | halo_hi
        p = sb.tile([128, F], f32)
        x = sb.tile([128, F], f32)
        q = sb.tile([128, F], f32)        # A @ p via recurrence
        t0 = sb.tile([128, F], f32)
        t1 = sb.tile([128, F], f32)
        t2 = sb.tile([128, F], f32)
        t3 = sb.tile([128, F], f32)
        cD = sb.tile([128, F], f32)
        cL = sb.tile([128, COLS * (M + 1)], f32)
        cB = sb.tile([128, (COLS + 1) * M], f32)
        bc128 = sb.tile([128, 128], f32)
        sdn = sb.tile([128, 128], f32)
        sup = sb.tile([128, 128], f32)

        pap_part = sb.tile([128, 1], f32)
        rr_part = sb.tile([128, 1], f32)
        gamvec = sb.tile([128, 1], f32)   # per-problem gamma, broadcast
        recg = sb.tile([128, 1], f32)     # 1/gamma_old
        recp = sb.tile([128, 1], f32)     # 1/pAp
        avec = sb.tile([128, 1], f32)
        bvec = sb.tile([128, 1], f32)

        pap_ps = ps.tile([128, 1], f32)
        gam_ps = ps.tile([128, 1], f32)
        hlo_ps = ps.tile([128, M], f32)
        hhi_ps = ps.tile([128, M], f32)

        def v3(ap2d):
            return ap2d.rearrange("p (c j) -> p c j", c=COLS, j=M)

        r_c2 = r[:, M:M + F]
        r_c3 = v3(r_c2)
        r_jm1 = v3(r[:, M - 1:M - 1 + F])
        r_jp1 = v3(r[:, M + 1:M + 1 + F])
        r_km1 = v3(r[:, 0:F])
        r_kp1 = v3(r[:, 2 * M:2 * M + F])
        cL3 = cL[:].rearrange("p (c j) -> p c j", c=COLS, j=M + 1)
        cLl = cL3[:, :, 0:M]
        cLr = cL3[:, :, 1:M + 1]
        cB3 = cB[:].rearrange("p (c j) -> p c j", c=COLS + 1, j=M)
        cBb = cB3[:, 0:COLS, :]
        cBt = cB3[:, 1:COLS + 1, :]
        cD3 = v3(cD[:])

        nc.sync.dma_start(cD[:], cD_d)
        nc.sync.dma_start(cL[:], cL_d)
        nc.sync.dma_start(cB[:], cB_d)
        nc.sync.dma_start(bc128[:], bc128_d)
        nc.sync.dma_start(sdn[:], sdn_d)
        nc.sync.dma_start(sup[:], sup_d)
        nc.sync.dma_start(r_c2, f_d)
        nc.sync.dma_start(p[:], f_d)

        def halo_update():
            nc.tensor.matmul(hlo_ps[:], sdn[:], r[:, F:F + M])
            nc.tensor.matmul(hhi_ps[:], sup[:], r[:, M:2 * M])
            nc.scalar.copy(r[:, 0:M], hlo_ps[:])
            nc.scalar.copy(r[:, F + M:F + 2 * M], hhi_ps[:])

        def stencil_w():
            """t0 = A @ r (j-terms on DVE, k-products on GpSimd)."""
            nc.gpsimd.tensor_mul(v3(t2[:]), cBb, r_km1)
            nc.gpsimd.tensor_mul(v3(t3[:]), cBt, r_kp1)
            nc.vector.tensor_mul(v3(t0[:]), cD3, r_c3)
            nc.vector.tensor_mul(v3(t1[:]), cLl, r_jm1)
            nc.vector.tensor_add(t0[:], t0[:], t1[:])
            nc.vector.tensor_mul(v3(t1[:]), cLr, r_jp1)
            nc.vector.tensor_add(t0[:], t0[:], t1[:])
            nc.vector.tensor_add(t0[:], t0[:], t2[:])
            nc.vector.tensor_add(t0[:], t0[:], t3[:])

        # ---- init: x=0, r=p=f, q = A p, gamma0
        nc.vector.memset(x[:], 0.0)
        halo_update()
        nc.scalar.activation(t1[:], r_c2, Act.Square, accum_out=rr_part[:])
        nc.tensor.matmul(gam_ps[:], bc128[:], rr_part[:])
        nc.scalar.copy(gamvec[:], gam_ps[:])
        stencil_w()
        nc.vector.tensor_copy(q[:], t0[:])

        # ---- 300 CG iterations
        with tc.For_i(0, iters) as _i:
            nc.vector.reciprocal(recg[:], gamvec[:])

            # pAp and alpha
            nc.vector.tensor_mul(t3[:], p[:], q[:])
            nc.scalar.activation(t3[:], t3[:], Act.Copy, accum_out=pap_part[:])
            nc.tensor.matmul(pap_ps[:], bc128[:], pap_part[:])
            nc.vector.reciprocal(recp[:], pap_ps[:])
            nc.vector.tensor_mul(avec[:], gamvec[:], recp[:])

            # x += alpha*p (ACT+GpSimd, off critical) ; r -= alpha*q (DVE)
            nc.scalar.activation(t2[:], p[:], Act.Copy, scale=avec[:])
            nc.gpsimd.tensor_add(x[:], x[:], t2[:])
            nc.vector.tensor_scalar_mul(t1[:], q[:], avec[:])
            nc.vector.tensor_sub(r_c2, r_c2, t1[:])
            halo_update()

            # gamma' and beta (hidden under the stencil)
            nc.scalar.activation(t1[:], r_c2, Act.Square, accum_out=rr_part[:])
            nc.tensor.matmul(gam_ps[:], bc128[:], rr_part[:])
            nc.vector.tensor_mul(bvec[:], gam_ps[:], recg[:])
            nc.scalar.copy(gamvec[:], gam_ps[:])

            # w = A r
            stencil_w()

            # p = r + beta*p (GpSimd) ; q = w + beta*q (DVE)
            nc.gpsimd.tensor_scalar_mul(t2[:], p[:], bvec[:])
            nc.gpsimd.tensor_add(p[:], r_c2, t2[:])
            nc.vector.tensor_scalar_mul(t1[:], q[:], bvec[:])
            nc.vector.tensor_add(q[:], t0[:], t1[:])

        nc.sync.dma_start(x_d, x[:])

    nc.compile()
    return nc


def _build_nc(iters):
    from contextlib import ExitStack
    import concourse.bass as bass
    import concourse.tile as tile
    from concourse import bacc, mybir

    f32 = mybir.dt.float32
    Alu = mybir.AluOpType
    Act = mybir.ActivationFunctionType

    nc = bacc.Bacc("TRN2", target_bir_lowering=False, debug=False)

    f_d = nc.dram_tensor("f_in", [128, F], f32, kind="ExternalInput").ap()
    cD_d = nc.dram_tensor("cD_in", [128, F], f32, kind="ExternalInput").ap()
    cL_d = nc.dram_tensor("cL_in", [128, COLS * (M + 1)], f32, kind="ExternalInput").ap()
    cB_d = nc.dram_tensor("cB_in", [128, (COLS + 1) * M], f32, kind="ExternalInput").ap()
    bc128_d = nc.dram_tensor("bc128_in", [128, 128], f32, kind="ExternalInput").ap()
    sdn_d = nc.dram_tensor("sdn_in", [128, 128], f32, kind="ExternalInput").ap()
    sup_d = nc.dram_tensor("sup_in", [128, 128], f32, kind="ExternalInput").ap()
    x_d = nc.dram_tensor("x_out", [128, F], f32, kind="ExternalOutput").ap()

    with tile.TileContext(nc) as tc, ExitStack() as ctx:
        sb = ctx.enter_context(tc.tile_pool(name="state", bufs=1))
        ps = ctx.enter_context(tc.tile_pool(name="psum", bufs=1, space="PSUM"))

        p = sb.tile([128, FH], f32)       # halo_lo | center | halo_hi
        r = sb.tile([128, F], f32)
        x = sb.tile([128, F], f32)
        q = sb.tile([128, F], f32)        # A @ p
        t0 = sb.tile([128, F], f32)
        t1 = sb.tile([128, F], f32)
        t2 = sb.tile([128, F], f32)
        t3 = sb.tile([128, F], f32)
        cD = sb.tile([128, F], f32)
        cL = sb.tile([128, COLS * (M + 1)], f32)
        cB = sb.tile([128, (COLS + 1) * M], f32)
        bc128 = sb.tile([128, 128], f32)
        sdn = sb.tile([128, 128], f32)
        sup = sb.tile([128, 128], f32)

        pap_part = sb.tile([128, 1], f32)
        rr_part = sb.tile([128, 1], f32)
        gamvec = sb.tile([128, 1], f32)   # per-problem gamma, broadcast
        recg = sb.tile([128, 1], f32)
        recp = sb.tile([128, 1], f32)
        avec = sb.tile([128, 1], f32)
        bvec = sb.tile([128, 1], f32)

        pap_ps = ps.tile([128, 1], f32)
        gam_ps = ps.tile([128, 1], f32)
        hlo_ps = ps.tile([128, M], f32)
        hhi_ps = ps.tile([128, M], f32)

        # 3D views [128, 16, 256] over the stencil operands
        def v3(ap2d):
            return ap2d.rearrange("p (c j) -> p c j", c=COLS, j=M)

        p_c2 = p[:, M:M + F]
        p_c3 = v3(p_c2)
        p_jm1 = v3(p[:, M - 1:M - 1 + F])
        p_jp1 = v3(p[:, M + 1:M + 1 + F])
        p_km1 = v3(p[:, 0:F])
        p_kp1 = v3(p[:, 2 * M:2 * M + F])
        cL3 = cL[:].rearrange("p (c j) -> p c j", c=COLS, j=M + 1)
        cLl = cL3[:, :, 0:M]        # multiplies p_jm1
        cLr = cL3[:, :, 1:M + 1]    # multiplies p_jp1 (= K_right view)
        cB3 = cB[:].rearrange("p (c j) -> p c j", c=COLS + 1, j=M)
        cBb = cB3[:, 0:COLS, :]     # multiplies p_km1
        cBt = cB3[:, 1:COLS + 1, :] # multiplies p_kp1 (= K_top view)
        cD3 = v3(cD[:])

        # ---- load inputs
        nc.sync.dma_start(cD[:], cD_d)
        nc.sync.dma_start(cL[:], cL_d)
        nc.sync.dma_start(cB[:], cB_d)
        nc.sync.dma_start(bc128[:], bc128_d)
        nc.sync.dma_start(sdn[:], sdn_d)
        nc.sync.dma_start(sup[:], sup_d)
        nc.sync.dma_start(r[:], f_d)
        nc.sync.dma_start(p_c2, f_d)

        def halo_update():
            # halo_lo[P] = center_last_col[P-1]; halo_hi[P] = center_first_col[P+1]
            nc.tensor.matmul(hlo_ps[:], sdn[:], p[:, F:F + M])
            nc.tensor.matmul(hhi_ps[:], sup[:], p[:, M:2 * M])
            nc.scalar.copy(p[:, 0:M], hlo_ps[:])
            nc.scalar.copy(p[:, F + M:F + 2 * M], hhi_ps[:])

        # ---- init: x=0, gamma0 = per-problem ||f||^2, p halos
        nc.vector.memset(x[:], 0.0)
        halo_update()
        nc.scalar.activation(t1[:], r[:], Act.Square, accum_out=rr_part[:])
        nc.tensor.matmul(gam_ps[:], bc128[:], rr_part[:])
        nc.scalar.copy(gamvec[:], gam_ps[:])

        # ---- 300 CG iterations
        loop_mode = os.environ.get("KERNEL_LOOP", "unroll4")

        def body(_i):
            # 1/gamma_old for beta, overlappable with the stencil
            nc.vector.reciprocal(recg[:], gamvec[:])

            # q = A @ p  (GpSimd: k-shift products; DVE: the rest)
            nc.gpsimd.tensor_mul(v3(t2[:]), cBb, p_km1)
            nc.gpsimd.tensor_mul(v3(t3[:]), cBt, p_kp1)
            nc.vector.tensor_mul(v3(t0[:]), cD3, p_c3)
            nc.vector.tensor_mul(v3(t1[:]), cLl, p_jm1)
            nc.vector.tensor_add(t0[:], t0[:], t1[:])
            nc.vector.tensor_mul(v3(t1[:]), cLr, p_jp1)
            nc.vector.tensor_add(t0[:], t0[:], t1[:])
            nc.vector.tensor_add(t0[:], t0[:], t2[:])
            nc.vector.tensor_add(q[:], t0[:], t3[:])

            # pAp and alpha, all on DVE (no cross-engine hops in the chain)
            nc.vector.tensor_mul(t3[:], p_c2, q[:])
            nc.vector.tensor_reduce(pap_part[:], t3[:], mybir.AxisListType.X,
                                    Alu.add)
            nc.tensor.matmul(pap_ps[:], bc128[:], pap_part[:])
            nc.vector.reciprocal(recp[:], pap_ps[:])
            nc.vector.tensor_mul(avec[:], gamvec[:], recp[:])

            # r -= alpha*q (DVE) ; x += alpha*p (ACT scale + GpSimd add)
            nc.vector.tensor_scalar_mul(t1[:], q[:], avec[:])
            nc.vector.tensor_sub(r[:], r[:], t1[:])
            nc.scalar.activation(t0[:], p_c2, Act.Copy, scale=avec[:])
            nc.gpsimd.tensor_add(x[:], x[:], t0[:])

            # gamma_new = ||r||^2 (ACT), beta = gamma_new / gamma_old
            nc.scalar.activation(t2[:], r[:], Act.Square, accum_out=rr_part[:])
            nc.tensor.matmul(gam_ps[:], bc128[:], rr_part[:])
            nc.vector.tensor_mul(bvec[:], gam_ps[:], recg[:])
            nc.scalar.copy(gamvec[:], gam_ps[:])

            # p = r + beta*p, then refresh halos via PE shifts
            nc.vector.tensor_scalar_mul(t1[:], p_c2, bvec[:])
            nc.vector.tensor_add(p_c2, r[:], t1[:])
            halo_update()

        if loop_mode == "plain":
            with tc.For_i(0, iters) as _i:
                body(_i)
        elif loop_mode == "stag":
            with tc.For_i(0, iters, staggered_reset=True) as _i:
                body(_i)
        elif loop_mode.startswith("unroll"):
            tc.For_i_unrolled(0, iters, 1, body, max_unroll=int(loop_mode[6:]))
        else:
            raise ValueError(loop_mode)

        nc.sync.dma_start(x_d, x[:])

    nc.compile()
    return nc


VARIANT = os.environ.get("KERNEL_VARIANT", "std")


def _get_nc(iters, variant=None):
    variant = variant or VARIANT
    key = ("nc", iters, variant, os.environ.get("KERNEL_LOOP", "unroll4"))
    if key not in _CACHE:
        builder = {"std": _build_nc, "qrec": _build_nc_qrec}[variant]
        _CACHE[key] = builder(iters)
    return _CACHE[key]


def _expected_inputs(nc):
    import concourse.mybir as mybir
    part = nc.partition_id_tensor.name if nc.partition_id_tensor else None
    names = set()
    for alloc in nc.m.functions[0].allocations:
        if isinstance(alloc, mybir.MemoryLocationSet) and alloc.kind == "ExternalInput":
            nm = alloc.memorylocations[0].name
            if nm != part:
                names.add(nm)
    return names


# ------------------------------------------------------------------- runner

def _make_runner(iters, variant=None):
    """Build the 8-core sharded jit once; returns run(in_maps) -> [x_out]*8."""
    import jax
    from jax.sharding import Mesh, PartitionSpec
    from jax.experimental.shard_map import shard_map
    from concourse import bass2jax, mybir

    nc = _get_nc(iters, variant)
    bass2jax.install_neuronx_cc_hook()
    partition_name = nc.partition_id_tensor.name if nc.partition_id_tensor else None
    in_names, out_names, out_avals, zero_outs = [], [], [], []
    for alloc in nc.m.functions[0].allocations:
        if not isinstance(alloc, mybir.MemoryLocationSet):
            continue
        name = alloc.memorylocations[0].name
        if alloc.kind == "ExternalInput":
            if name != partition_name:
                in_names.append(name)
        elif alloc.kind == "ExternalOutput":
            out_names.append(name)
            shape = tuple(alloc.tensor_shape)
            dtype = mybir.dt.np(alloc.dtype)
            out_avals.append(jax.core.ShapedArray(shape, dtype))
            zero_outs.append(np.zeros(shape, dtype))
    n_params = len(in_names)
    all_in = in_names + out_names + ([partition_name] if partition_name else [])

    def _body(*args):
        ops = list(args)
        if partition_name:
            ops.append(bass2jax.partition_id_tensor())
        return tuple(bass2jax._bass_exec_p.bind(
            *ops, out_avals=tuple(out_avals), in_names=tuple(all_in),
            out_names=tuple(out_names), lowering_input_output_aliases=(),
            sim_require_finite=True, sim_require_nnan=True, nc=nc))

    mesh = Mesh(np.asarray(jax.devices()[:NCORES]), ("core",))
    jf = jax.jit(
        shard_map(_body, mesh=mesh,
                  in_specs=(PartitionSpec("core"),) * (n_params + len(out_names)),
                  out_specs=(PartitionSpec("core"),) * len(out_names),
                  check_rep=False),
        donate_argnums=tuple(range(n_params, n_params + len(out_names))),
        keep_unused=True)

    def prepare(in_maps):
        import jax
        concat_in = [np.concatenate([m[nm] for m in in_maps], axis=0)
                     for nm in in_names]
        dev_in = [jax.device_put(a) for a in concat_in]
        jax.block_until_ready(dev_in)
        return dev_in

    def run_dev(dev_in, fetch=True):
        import jax
        zeros = [np.zeros((NCORES * z.shape[0], *z.shape[1:]), z.dtype)
                 for z in zero_outs]
        outs = jf(*dev_in, *zeros)
        if not fetch:
            jax.block_until_ready(outs)
            return None
        xo = np.asarray(outs[out_names.index("x_out")])
        per_core_rows = xo.shape[0] // NCORES
        return [xo[c * per_core_rows:(c + 1) * per_core_rows] for c in range(NCORES)]

    def run(in_maps):
        return run_dev(prepare(in_maps))

    run.prepare = prepare
    run.run_dev = run_dev
    return run


def _get_runner(iters, variant=None):
    variant = variant or VARIANT
    key = ("runner", iters, variant, os.environ.get("KERNEL_LOOP", "unroll4"))
    if key not in _CACHE:
        _CACHE[key] = _make_runner(iters, variant)
    return _CACHE[key]


def _run(in_maps, iters, variant=None):
    return _get_runner(iters, variant)(in_maps)


def kernel(alpha, f_rhs):
    alpha = np.asarray(alpha, np.float32)
    f_rhs = np.asarray(f_rhs, np.float32)
    in_maps = [_pack_core(alpha[c * BPC:(c + 1) * BPC], f_rhs)
               for c in range(NCORES)]
    try:
        outs = _run(in_maps, ITERS)
    except Exception:
        # a crashed prior session can leave a core wedged; one retry clears it
        outs = _run(in_maps, ITERS)
    return np.concatenate([_from_dev(o) for o in outs], axis=0)


# revision 25
# speedup vs baseline: 81.9333x; 1.0541x over previous
"""Trainium2 Bass kernel for batched 2D variable-coefficient diffusion CG solve.

Problem: 64 independent solves of A(alpha) u = f_rhs on a 256x256 grid,
5-point stencil with edge coefficients exp(0.5*(alpha_a + alpha_b)), solved
with 300 fp32 CG iterations (the reference's jax CG never converges before
maxiter=300 at tol=1e-6 in fp32, so the output is exactly the 300th iterate).

Sharding: pure data parallel, 8 problems per NeuronCore across 8 cores.

Per-core layout: partition P = b*16 + kb (b = local problem 0..7, kb = k-block
0..15); each partition holds 16 k-columns x 256 j-rows, free index = c*256 + j
for k = kb*16 + c. All CG state lives in SBUF for all 300 iterations; the only
cross-partition traffic is a 1-column halo per side for the k-direction stencil
shifts, done with two tiny TensorE shift matmuls per iteration.

Engine split (custom fused DVE ops crash under this runtime, so native ops
only): DVE does the j-direction stencil products + all sums, the <p,Ap>
product + L->R reduce, and the r/p axpys (tensor_scalar at 2x + tensor add);
GpSimd runs the two k-direction stencil products and the x update in
parallel; ACT does the x scale and the ||r||^2 square-accumulate; PE does the
block-diagonal ones-matmul that both segment-sums the 16 per-partition dot
partials of each problem and broadcasts the result back to its partitions,
plus the two 1-column halo shift matmuls. The 300-iteration loop is a
hardware For_i unrolled x4 (the all-engine back-edge barrier costs ~7us).

Measured on trn2 (8 cores): ~82 us/iteration -> ~25 ms device time for the
full solve; output matches the CPU jax reference at 1.5e-2 absmax relative
(the fp32 reproducibility envelope of this unconverged CG trajectory:
independent fp32 implementations of the same algorithm differ by ~1e-2).
"""

import os
import numpy as np

M = 256
B = 64
NCORES = 8
BPC = B // NCORES          # problems per core
HINV2 = np.float32(M * M)  # exact power of two: folding into coeffs is exact
ITERS = 300
COLS = 16                  # k-columns per partition
F = COLS * M               # 4096 free elements per field
FH = F + 2 * M             # p buffer with halo columns

_CACHE = {}


# ----------------------------------------------------------------- host side

def _coeff_arrays(alpha):
    """Per-problem stencil coefficient fields, matching reference._stencil_coeffs
    fp32 op-for-op, with HINV2 folded in (exact) and off-diagonals negated.

    alpha: (B, 257, 257) f32. Returns diag, KL, KB as (B, 256, 256) f32 where
    KL/KB are the *unmasked-left* / *masked-bottom* edge coefficients."""
    a = alpha.astype(np.float32)
    m = M
    j = np.arange(m)[:, None]
    k = np.arange(m)[None, :]
    KL = np.exp(np.float32(0.5) * (a[:, :-1, :-1] + a[:, :-1, 1:])).astype(np.float32)
    KR = np.where(j < m - 1,
                  np.exp(np.float32(0.5) * (a[:, 1:, :-1] + a[:, 1:, 1:])),
                  np.float32(0.0)).astype(np.float32)
    KB = np.where(k > 0,
                  np.exp(np.float32(0.5) * (a[:, :-1, :-1] + a[:, 1:, :-1])),
                  np.float32(0.0)).astype(np.float32)
    KT = np.where(k < m - 1,
                  np.exp(np.float32(0.5) * (a[:, :-1, 1:] + a[:, 1:, 1:])),
                  np.float32(0.0)).astype(np.float32)
    diag = KL + KR + KB + KT + np.where(j == 0, KL, np.float32(0.0)).astype(np.float32)
    return diag, KL, KB


def _to_dev(arr_bjk):
    """(BPC, 256j, 256k) -> [128, 4096] with P = b*16+kb, free = c*256+j."""
    t = arr_bjk.transpose(0, 2, 1)                 # (b, k, j)
    t = t.reshape(BPC, 16, COLS, M)                # (b, kb, c, j)
    return np.ascontiguousarray(t.reshape(128, F))


def _from_dev(dev):
    """[128, 4096] -> (BPC, 256j, 256k)."""
    t = dev.reshape(BPC, 16, COLS, M).transpose(0, 3, 1, 2)   # (b, j, kb, c)
    return np.ascontiguousarray(t.reshape(BPC, M, M))


def _pack_core(alpha_core, f_rhs):
    """Build the per-core input map (all fp32 numpy arrays)."""
    diag, KL, KB = _coeff_arrays(alpha_core)
    s = HINV2
    cD = _to_dev(diag * s)                               # [128, 4096]
    nKL = _to_dev(KL * (-s)).reshape(128, COLS, M)       # (P, c, j)
    nKB = _to_dev(KB * (-s)).reshape(128, COLS, M)

    # cLp[P, c, 0..256]: 0 at jj=0 (Dirichlet kill for the j-1 shift),
    # -s*KL[jj,k] at jj=1..255, 0 at jj=256 (K_right mask at j=255).
    cLp = np.zeros((128, COLS, M + 1), np.float32)
    cLp[:, :, 1:M] = nKL[:, :, 1:M]

    # cBp[P, 0..16, j]: c=0..15 the (already k-masked) bottom coefficients,
    # c=16 the next partition's c=0 column (static k-halo; 0 past k=255).
    cBp = np.zeros((128, COLS + 1, M), np.float32)
    cBp[:, :COLS, :] = nKB
    nKB4 = nKB.reshape(BPC, 16, COLS, M)
    cBp4 = cBp.reshape(BPC, 16, COLS + 1, M)
    cBp4[:, :-1, COLS, :] = nKB4[:, 1:, 0, :]

    fdev = _to_dev(np.broadcast_to(f_rhs, (BPC, M, M)).astype(np.float32))

    seg = np.zeros((128, BPC), np.float32)               # seg[q, b] = q//16 == b
    seg[np.arange(128), np.arange(128) // 16] = 1.0
    bc = np.ascontiguousarray(seg.T)                     # (8, 128)
    qi = np.arange(128)
    bc128 = (qi[:, None] // 16 == qi[None, :] // 16).astype(np.float32)
    sdn = np.eye(128, 128, 1, np.float32)                # out[i] = in[i-1]
    sup = np.eye(128, 128, -1, np.float32)               # out[i] = in[i+1]

    return {
        "f_in": fdev,
        "cD_in": cD,
        "cL_in": np.ascontiguousarray(cLp.reshape(128, COLS * (M + 1))),
        "cB_in": np.ascontiguousarray(cBp.reshape(128, (COLS + 1) * M)),
        "seg_in": seg,
        "bc_in": bc,
        "bc128_in": bc128,
        "sdn_in": sdn,
        "sup_in": sup,
    }


# --------------------------------------------------------------- bass kernel

def _build_nc_qrec(iters):
    """q-recurrence variant: q_{k+1} = A r_{k+1} + beta_k q_k.

    The stencil runs on r (available right after the r update), so the
    ||r||^2 / beta / p-update chain hides behind it. Validated in exp3.py:
    lands as close to the f64 trajectory as plain fp32 CG.

    Loop state: p, q (= A p), r (halo'd), x, gamvec ([128,1] per-problem
    gamma broadcast). Body:
        pAp = <p, q>; alpha = gamma/pAp
        x += alpha p ; r -= alpha q ; refresh r halos
        gamma' = ||r||^2 ; beta = gamma'/gamma
        w = A r  (overlaps beta chain and p update)
        p = r + beta p ; q = w + beta q
    """
    from contextlib import ExitStack
    import concourse.bass as bass
    import concourse.tile as tile
    from concourse import bacc, mybir

    f32 = mybir.dt.float32
    Alu = mybir.AluOpType
    Act = mybir.ActivationFunctionType

    nc = bacc.Bacc("TRN2", target_bir_lowering=False, debug=False)

    f_d = nc.dram_tensor("f_in", [128, F], f32, kind="ExternalInput").ap()
    cD_d = nc.dram_tensor("cD_in", [128, F], f32, kind="ExternalInput").ap()
    cL_d = nc.dram_tensor("cL_in", [128, COLS * (M + 1)], f32, kind="ExternalInput").ap()
    cB_d = nc.dram_tensor("cB_in", [128, (COLS + 1) * M], f32, kind="ExternalInput").ap()
    bc128_d = nc.dram_tensor("bc128_in", [128, 128], f32, kind="ExternalInput").ap()
    sdn_d = nc.dram_tensor("sdn_in", [128, 128], f32, kind="ExternalInput").ap()
    sup_d = nc.dram_tensor("sup_in", [128, 128], f32, kind="ExternalInput").ap()
    x_d = nc.dram_tensor("x_out", [128, F], f32, kind="ExternalOutput").ap()

    with tile.TileContext(nc) as tc, ExitStack() as ctx:
        sb = ctx.enter_context(tc.tile_pool(name="state", bufs=1))
        ps = ctx.enter_context(tc.tile_pool(name="psum", bufs=1, space="PSUM"))

        r = sb.tile([128, FH], f32)       # halo_lo | center | halo_hi
        p = sb.tile([128, F], f32)
        x = sb.tile([128, F], f32)
        q = sb.tile([128, F], f32)        # A @ p via recurrence
        t0 = sb.tile([128, F], f32)
        t1 = sb.tile([128, F], f32)
        t2 = sb.tile([128, F], f32)
        t3 = sb.tile([128, F], f32)
        cD = sb.tile([128, F], f32)
        cL = sb.tile([128, COLS * (M + 1)], f32)
        cB = sb.tile([128, (COLS + 1) * M], f32)
        bc128 = sb.tile([128, 128], f32)
        sdn = sb.tile([128, 128], f32)
        sup = sb.tile([128, 128], f32)

        pap_part = sb.tile([128, 1], f32)
        rr_part = sb.tile([128, 1], f32)
        gamvec = sb.tile([128, 1], f32)   # per-problem gamma, broadcast
        recg = sb.tile([128, 1], f32)     # 1/gamma_old
        recp = sb.tile([128, 1], f32)     # 1/pAp
        avec = sb.tile([128, 1], f32)
        bvec = sb.tile([128, 1], f32)

        pap_ps = ps.tile([128, 1], f32)
        gam_ps = ps.tile([128, 1], f32)
        hlo_ps = ps.tile([128, M], f32)
        hhi_ps = ps.tile([128, M], f32)

        def v3(ap2d):
            return ap2d.rearrange("p (c j) -> p c j", c=COLS, j=M)

        r_c2 = r[:, M:M + F]
        r_c3 = v3(r_c2)
        r_jm1 = v3(r[:, M - 1:M - 1 + F])
        r_jp1 = v3(r[:, M + 1:M + 1 + F])
        r_km1 = v3(r[:, 0:F])
        r_kp1 = v3(r[:, 2 * M:2 * M + F])
        cL3 = cL[:].rearrange("p (c j) -> p c j", c=COLS, j=M + 1)
        cLl = cL3[:, :, 0:M]
        cLr = cL3[:, :, 1:M + 1]
        cB3 = cB[:].rearrange("p (c j) -> p c j", c=COLS + 1, j=M)
        cBb = cB3[:, 0:COLS, :]
        cBt = cB3[:, 1:COLS + 1, :]
        cD3 = v3(cD[:])

        nc.sync.dma_start(cD[:], cD_d)
        nc.sync.dma_start(cL[:], cL_d)
        nc.sync.dma_start(cB[:], cB_d)
        nc.sync.dma_start(bc128[:], bc128_d)
        nc.sync.dma_start(sdn[:], sdn_d)
        nc.sync.dma_start(sup[:], sup_d)
        nc.sync.dma_start(r_c2, f_d)
        nc.sync.dma_start(p[:], f_d)

        def halo_update():
            nc.tensor.matmul(hlo_ps[:], sdn[:], r[:, F:F + M])
            nc.tensor.matmul(hhi_ps[:], sup[:], r[:, M:2 * M])
            nc.scalar.copy(r[:, 0:M], hlo_ps[:])
            nc.scalar.copy(r[:, F + M:F + 2 * M], hhi_ps[:])

        def stencil_w():
            """t0 = A @ r (j-terms on DVE, k-products on GpSimd)."""
            nc.gpsimd.tensor_mul(v3(t2[:]), cBb, r_km1)
            nc.gpsimd.tensor_mul(v3(t3[:]), cBt, r_kp1)
            nc.vector.tensor_mul(v3(t0[:]), cD3, r_c3)
            nc.vector.tensor_mul(v3(t1[:]), cLl, r_jm1)
            nc.vector.tensor_add(t0[:], t0[:], t1[:])
            nc.vector.tensor_mul(v3(t1[:]), cLr, r_jp1)
            nc.vector.tensor_add(t0[:], t0[:], t1[:])
            nc.vector.tensor_add(t0[:], t0[:], t2[:])
            nc.vector.tensor_add(t0[:], t0[:], t3[:])

        # ---- init: x=0, r=p=f, q = A p, gamma0
        nc.vector.memset(x[:], 0.0)
        halo_update()
        nc.scalar.activation(t1[:], r_c2, Act.Square, accum_out=rr_part[:])
        nc.tensor.matmul(gam_ps[:], bc128[:], rr_part[:])
        nc.scalar.copy(gamvec[:], gam_ps[:])
        stencil_w()
        nc.vector.tensor_copy(q[:], t0[:])

        # ---- 300 CG iterations
        with tc.For_i(0, iters) as _i:
            nc.vector.reciprocal(recg[:], gamvec[:])

            # pAp and alpha
            nc.vector.tensor_mul(t3[:], p[:], q[:])
            nc.scalar.activation(t3[:], t3[:], Act.Copy, accum_out=pap_part[:])
            nc.tensor.matmul(pap_ps[:], bc128[:], pap_part[:])
            nc.vector.reciprocal(recp[:], pap_ps[:])
            nc.vector.tensor_mul(avec[:], gamvec[:], recp[:])

            # x += alpha*p (ACT+GpSimd, off critical) ; r -= alpha*q (DVE)
            nc.scalar.activation(t2[:], p[:], Act.Copy, scale=avec[:])
            nc.gpsimd.tensor_add(x[:], x[:], t2[:])
            nc.vector.tensor_scalar_mul(t1[:], q[:], avec[:])
            nc.vector.tensor_sub(r_c2, r_c2, t1[:])
            halo_update()

            # gamma' and beta (hidden under the stencil)
            nc.scalar.activation(t1[:], r_c2, Act.Square, accum_out=rr_part[:])
            nc.tensor.matmul(gam_ps[:], bc128[:], rr_part[:])
            nc.vector.tensor_mul(bvec[:], gam_ps[:], recg[:])
            nc.scalar.copy(gamvec[:], gam_ps[:])

            # w = A r
            stencil_w()

            # p = r + beta*p (GpSimd) ; q = w + beta*q (DVE)
            nc.gpsimd.tensor_scalar_mul(t2[:], p[:], bvec[:])
            nc.gpsimd.tensor_add(p[:], r_c2, t2[:])
            nc.vector.tensor_scalar_mul(t1[:], q[:], bvec[:])
            nc.vector.tensor_add(q[:], t0[:], t1[:])

        nc.sync.dma_start(x_d, x[:])

    nc.compile()
    return nc


def _build_nc(iters):
    from contextlib import ExitStack
    import concourse.bass as bass
    import concourse.tile as tile
    from concourse import bacc, mybir

    f32 = mybir.dt.float32
    Alu = mybir.AluOpType
    Act = mybir.ActivationFunctionType

    nc = bacc.Bacc("TRN2", target_bir_lowering=False, debug=False)

    f_d = nc.dram_tensor("f_in", [128, F], f32, kind="ExternalInput").ap()
    cD_d = nc.dram_tensor("cD_in", [128, F], f32, kind="ExternalInput").ap()
    cL_d = nc.dram_tensor("cL_in", [128, COLS * (M + 1)], f32, kind="ExternalInput").ap()
    cB_d = nc.dram_tensor("cB_in", [128, (COLS + 1) * M], f32, kind="ExternalInput").ap()
    bc128_d = nc.dram_tensor("bc128_in", [128, 128], f32, kind="ExternalInput").ap()
    sdn_d = nc.dram_tensor("sdn_in", [128, 128], f32, kind="ExternalInput").ap()
    sup_d = nc.dram_tensor("sup_in", [128, 128], f32, kind="ExternalInput").ap()
    x_d = nc.dram_tensor("x_out", [128, F], f32, kind="ExternalOutput").ap()

    with tile.TileContext(nc) as tc, ExitStack() as ctx:
        sb = ctx.enter_context(tc.tile_pool(name="state", bufs=1))
        ps = ctx.enter_context(tc.tile_pool(name="psum", bufs=1, space="PSUM"))

        p = sb.tile([128, FH], f32)       # halo_lo | center | halo_hi
        r = sb.tile([128, F], f32)
        x = sb.tile([128, F], f32)
        q = sb.tile([128, F], f32)        # A @ p
        t0 = sb.tile([128, F], f32)       # DVE stencil accumulator
        t1 = sb.tile([128, F], f32)       # DVE-only scratch (products, axpy terms)
        t2 = sb.tile([128, F], f32)       # GpSimd m3 product / ACT rr junk
        t3 = sb.tile([128, F], f32)       # GpSimd m4 product / pAp product / x term
        t4 = sb.tile([128, F], f32)       # GpSimd m1 product (dedicated)
        cD = sb.tile([128, F], f32)
        cL = sb.tile([128, COLS * (M + 1)], f32)
        cB = sb.tile([128, (COLS + 1) * M], f32)
        bc128 = sb.tile([128, 128], f32)
        sdn = sb.tile([128, 128], f32)
        sup = sb.tile([128, 128], f32)

        pap_part = sb.tile([128, 1], f32)
        rr_part = sb.tile([128, 1], f32)
        gamvec = sb.tile([128, 1], f32)   # per-problem gamma, broadcast
        recg = sb.tile([128, 1], f32)
        recp = sb.tile([128, 1], f32)
        avec = sb.tile([128, 1], f32)
        aneg = sb.tile([128, 1], f32)
        bvec = sb.tile([128, 1], f32)

        pap_ps = ps.tile([128, 1], f32)
        gam_ps = ps.tile([128, 1], f32)
        hlo_ps = ps.tile([128, M], f32)
        hhi_ps = ps.tile([128, M], f32)

        # 3D views [128, 16, 256] over the stencil operands
        def v3(ap2d):
            return ap2d.rearrange("p (c j) -> p c j", c=COLS, j=M)

        p_c2 = p[:, M:M + F]
        p_c3 = v3(p_c2)
        p_jm1 = v3(p[:, M - 1:M - 1 + F])
        p_jp1 = v3(p[:, M + 1:M + 1 + F])
        p_km1 = v3(p[:, 0:F])
        p_kp1 = v3(p[:, 2 * M:2 * M + F])
        cL3 = cL[:].rearrange("p (c j) -> p c j", c=COLS, j=M + 1)
        cLl = cL3[:, :, 0:M]        # multiplies p_jm1
        cLr = cL3[:, :, 1:M + 1]    # multiplies p_jp1 (= K_right view)
        cB3 = cB[:].rearrange("p (c j) -> p c j", c=COLS + 1, j=M)
        cBb = cB3[:, 0:COLS, :]     # multiplies p_km1
        cBt = cB3[:, 1:COLS + 1, :] # multiplies p_kp1 (= K_top view)
        cD3 = v3(cD[:])

        # ---- load inputs
        nc.sync.dma_start(cD[:], cD_d)
        nc.sync.dma_start(cL[:], cL_d)
        nc.sync.dma_start(cB[:], cB_d)
        nc.sync.dma_start(bc128[:], bc128_d)
        nc.sync.dma_start(sdn[:], sdn_d)
        nc.sync.dma_start(sup[:], sup_d)
        nc.sync.dma_start(r[:], f_d)
        nc.sync.dma_start(p_c2, f_d)

        def halo_update():
            # halo_lo[P] = center_last_col[P-1]; halo_hi[P] = center_first_col[P+1]
            nc.tensor.matmul(hlo_ps[:], sdn[:], p[:, F:F + M])
            nc.tensor.matmul(hhi_ps[:], sup[:], p[:, M:2 * M])
            nc.scalar.copy(p[:, 0:M], hlo_ps[:])
            nc.scalar.copy(p[:, F + M:F + 2 * M], hhi_ps[:])

        # ---- init: x=0, gamma0 = per-problem ||f||^2, p halos
        nc.vector.memset(x[:], 0.0)
        halo_update()
        nc.scalar.activation(t1[:], r[:], Act.Square, accum_out=rr_part[:])
        nc.tensor.matmul(gam_ps[:], bc128[:], rr_part[:])
        nc.scalar.copy(gamvec[:], gam_ps[:])

        # ---- 300 CG iterations
        loop_mode = os.environ.get("KERNEL_LOOP", "unroll4")

        def body(_i):
            # 1/gamma_old for beta, overlappable with the stencil
            nc.vector.reciprocal(recg[:], gamvec[:])

            # q = A @ p  (GpSimd: k-shift products; DVE: the rest)
            nc.gpsimd.tensor_mul(v3(t2[:]), cBb, p_km1)
            nc.gpsimd.tensor_mul(v3(t3[:]), cBt, p_kp1)
            nc.vector.tensor_mul(v3(t0[:]), cD3, p_c3)
            nc.vector.tensor_mul(v3(t1[:]), cLl, p_jm1)
            nc.vector.tensor_add(t0[:], t0[:], t1[:])
            nc.vector.tensor_mul(v3(t1[:]), cLr, p_jp1)
            nc.vector.tensor_add(t0[:], t0[:], t1[:])
            nc.vector.tensor_add(t0[:], t0[:], t2[:])
            nc.vector.tensor_add(q[:], t0[:], t3[:])

            # pAp = sum(p*q) fused in one DVE pass; alpha = gamma/pAp
            nc.vector.scalar_tensor_tensor(
                t3[:], p_c2, 1.0, q[:], Alu.mult, Alu.mult,
                accum_out=pap_part[:])
            nc.tensor.matmul(pap_ps[:], bc128[:], pap_part[:])
            nc.vector.reciprocal(recp[:], pap_ps[:])
            nc.vector.tensor_mul(avec[:], gamvec[:], recp[:])
            nc.vector.tensor_scalar_mul(aneg[:], avec[:], -1.0)

            # r = (q * -alpha) + r, one pass; x += alpha*p off-critical
            nc.vector.scalar_tensor_tensor(
                r[:], q[:], aneg[:], r[:], Alu.mult, Alu.add)
            nc.scalar.activation(t3[:], p_c2, Act.Copy, scale=avec[:])
            nc.gpsimd.tensor_add(x[:], x[:], t3[:])

            # gamma' = sum(r*r) fused on DVE (no engine hop); beta
            nc.vector.scalar_tensor_tensor(
                t2[:], r[:], 1.0, r[:], Alu.mult, Alu.mult,
                accum_out=rr_part[:])
            nc.tensor.matmul(gam_ps[:], bc128[:], rr_part[:])
            nc.vector.tensor_mul(bvec[:], gam_ps[:], recg[:])
            nc.scalar.copy(gamvec[:], gam_ps[:])

            # p = (p * beta) + r in one pass, then refresh halos
            nc.vector.scalar_tensor_tensor(
                p_c2, p_c2, bvec[:], r[:], Alu.mult, Alu.add)
            halo_update()

        if loop_mode == "plain":
            with tc.For_i(0, iters) as _i:
                body(_i)
        elif loop_mode == "stag":
            with tc.For_i(0, iters, staggered_reset=True) as _i:
                body(_i)
        elif loop_mode.startswith("unroll"):
            tc.For_i_unrolled(0, iters, 1, body, max_unroll=int(loop_mode[6:]))
        else:
            raise ValueError(loop_mode)

        nc.sync.dma_start(x_d, x[:])

    nc.compile()
    return nc


VARIANT = os.environ.get("KERNEL_VARIANT", "std")


def _get_nc(iters, variant=None):
    variant = variant or VARIANT
    key = ("nc", iters, variant, os.environ.get("KERNEL_LOOP", "unroll4"))
    if key not in _CACHE:
        builder = {"std": _build_nc, "qrec": _build_nc_qrec}[variant]
        _CACHE[key] = builder(iters)
    return _CACHE[key]


def _expected_inputs(nc):
    import concourse.mybir as mybir
    part = nc.partition_id_tensor.name if nc.partition_id_tensor else None
    names = set()
    for alloc in nc.m.functions[0].allocations:
        if isinstance(alloc, mybir.MemoryLocationSet) and alloc.kind == "ExternalInput":
            nm = alloc.memorylocations[0].name
            if nm != part:
                names.add(nm)
    return names


# ------------------------------------------------------------------- runner

def _make_runner(iters, variant=None):
    """Build the 8-core sharded jit once; returns run(in_maps) -> [x_out]*8."""
    import jax
    from jax.sharding import Mesh, PartitionSpec
    from jax.experimental.shard_map import shard_map
    from concourse import bass2jax, mybir

    nc = _get_nc(iters, variant)
    bass2jax.install_neuronx_cc_hook()
    partition_name = nc.partition_id_tensor.name if nc.partition_id_tensor else None
    in_names, out_names, out_avals, zero_outs = [], [], [], []
    for alloc in nc.m.functions[0].allocations:
        if not isinstance(alloc, mybir.MemoryLocationSet):
            continue
        name = alloc.memorylocations[0].name
        if alloc.kind == "ExternalInput":
            if name != partition_name:
                in_names.append(name)
        elif alloc.kind == "ExternalOutput":
            out_names.append(name)
            shape = tuple(alloc.tensor_shape)
            dtype = mybir.dt.np(alloc.dtype)
            out_avals.append(jax.core.ShapedArray(shape, dtype))
            zero_outs.append(np.zeros(shape, dtype))
    n_params = len(in_names)
    all_in = in_names + out_names + ([partition_name] if partition_name else [])

    def _body(*args):
        ops = list(args)
        if partition_name:
            ops.append(bass2jax.partition_id_tensor())
        return tuple(bass2jax._bass_exec_p.bind(
            *ops, out_avals=tuple(out_avals), in_names=tuple(all_in),
            out_names=tuple(out_names), lowering_input_output_aliases=(),
            sim_require_finite=True, sim_require_nnan=True, nc=nc))

    mesh = Mesh(np.asarray(jax.devices()[:NCORES]), ("core",))
    jf = jax.jit(
        shard_map(_body, mesh=mesh,
                  in_specs=(PartitionSpec("core"),) * (n_params + len(out_names)),
                  out_specs=(PartitionSpec("core"),) * len(out_names),
                  check_rep=False),
        donate_argnums=tuple(range(n_params, n_params + len(out_names))),
        keep_unused=True)

    def prepare(in_maps):
        import jax
        concat_in = [np.concatenate([m[nm] for m in in_maps], axis=0)
                     for nm in in_names]
        dev_in = [jax.device_put(a) for a in concat_in]
        jax.block_until_ready(dev_in)
        return dev_in

    def run_dev(dev_in, fetch=True):
        import jax
        zeros = [np.zeros((NCORES * z.shape[0], *z.shape[1:]), z.dtype)
                 for z in zero_outs]
        outs = jf(*dev_in, *zeros)
        if not fetch:
            jax.block_until_ready(outs)
            return None
        xo = np.asarray(outs[out_names.index("x_out")])
        per_core_rows = xo.shape[0] // NCORES
        return [xo[c * per_core_rows:(c + 1) * per_core_rows] for c in range(NCORES)]

    def run(in_maps):
        return run_dev(prepare(in_maps))

    run.prepare = prepare
    run.run_dev = run_dev
    return run


def _get_runner(iters, variant=None):
    variant = variant or VARIANT
    key = ("runner", iters, variant, os.environ.get("KERNEL_LOOP", "unroll4"))
    if key not in _CACHE:
        _CACHE[key] = _make_runner(iters, variant)
    return _CACHE[key]


def _run(in_maps, iters, variant=None):
    return _get_runner(iters, variant)(in_maps)


def kernel(alpha, f_rhs):
    alpha = np.asarray(alpha, np.float32)
    f_rhs = np.asarray(f_rhs, np.float32)
    in_maps = [_pack_core(alpha[c * BPC:(c + 1) * BPC], f_rhs)
               for c in range(NCORES)]
    try:
        outs = _run(in_maps, ITERS)
    except Exception:
        # a crashed prior session can leave a core wedged; one retry clears it
        outs = _run(in_maps, ITERS)
    return np.concatenate([_from_dev(o) for o in outs], axis=0)


# revision 30
# speedup vs baseline: 84.3289x; 1.0292x over previous
"""Trainium2 Bass kernel for batched 2D variable-coefficient diffusion CG solve.

Problem: 64 independent solves of A(alpha) u = f_rhs on a 256x256 grid,
5-point stencil with edge coefficients exp(0.5*(alpha_a + alpha_b)), solved
with 300 fp32 CG iterations (the reference's jax CG never converges before
maxiter=300 at tol=1e-6 in fp32, so the output is exactly the 300th iterate).

Sharding: pure data parallel, 8 problems per NeuronCore across 8 cores.

Per-core layout: partition P = b*16 + kb (b = local problem 0..7, kb = k-block
0..15); each partition holds 16 k-columns x 256 j-rows, free index = c*256 + j
for k = kb*16 + c. All CG state lives in SBUF for all 300 iterations; the only
cross-partition traffic is a 1-column halo per side for the k-direction stencil
shifts, done with two tiny TensorE shift matmuls per iteration.

Engine split (custom fused DVE ops crash under this runtime, so native ops
only): DVE does the j-direction stencil products + all sums, the <p,Ap>
product + L->R reduce, and the r/p axpys (tensor_scalar at 2x + tensor add);
GpSimd runs the two k-direction stencil products and the x update in
parallel; ACT does the x scale and the ||r||^2 square-accumulate; PE does the
block-diagonal ones-matmul that both segment-sums the 16 per-partition dot
partials of each problem and broadcasts the result back to its partitions,
plus the two 1-column halo shift matmuls. The 300-iteration loop is a
hardware For_i unrolled x4 (the all-engine back-edge barrier costs ~7us).

Measured on trn2 (8 cores): ~78 us/iteration -> ~23.5 ms device time for the
full solve; output matches the CPU jax reference at 1.7e-2 absmax relative
(the fp32 reproducibility envelope of this unconverged CG trajectory:
independent fp32 implementations of the same algorithm differ by ~1e-2).
"""

import os
import numpy as np

M = 256
B = 64
NCORES = 8
BPC = B // NCORES          # problems per core
HINV2 = np.float32(M * M)  # exact power of two: folding into coeffs is exact
ITERS = 300
COLS = 16                  # k-columns per partition
F = COLS * M               # 4096 free elements per field
FH = F + 2 * M             # p buffer with halo columns

_CACHE = {}


# ----------------------------------------------------------------- host side

def _coeff_arrays(alpha):
    """Per-problem stencil coefficient fields, matching reference._stencil_coeffs
    fp32 op-for-op, with HINV2 folded in (exact) and off-diagonals negated.

    alpha: (B, 257, 257) f32. Returns diag, KL, KB as (B, 256, 256) f32 where
    KL/KB are the *unmasked-left* / *masked-bottom* edge coefficients."""
    a = alpha.astype(np.float32)
    m = M
    j = np.arange(m)[:, None]
    k = np.arange(m)[None, :]
    KL = np.exp(np.float32(0.5) * (a[:, :-1, :-1] + a[:, :-1, 1:])).astype(np.float32)
    KR = np.where(j < m - 1,
                  np.exp(np.float32(0.5) * (a[:, 1:, :-1] + a[:, 1:, 1:])),
                  np.float32(0.0)).astype(np.float32)
    KB = np.where(k > 0,
                  np.exp(np.float32(0.5) * (a[:, :-1, :-1] + a[:, 1:, :-1])),
                  np.float32(0.0)).astype(np.float32)
    KT = np.where(k < m - 1,
                  np.exp(np.float32(0.5) * (a[:, :-1, 1:] + a[:, 1:, 1:])),
                  np.float32(0.0)).astype(np.float32)
    diag = KL + KR + KB + KT + np.where(j == 0, KL, np.float32(0.0)).astype(np.float32)
    return diag, KL, KB


def _to_dev(arr_bjk):
    """(BPC, 256j, 256k) -> [128, 4096] with P = b*16+kb, free = c*256+j."""
    t = arr_bjk.transpose(0, 2, 1)                 # (b, k, j)
    t = t.reshape(BPC, 16, COLS, M)                # (b, kb, c, j)
    return np.ascontiguousarray(t.reshape(128, F))


def _from_dev(dev):
    """[128, 4096] -> (BPC, 256j, 256k)."""
    t = dev.reshape(BPC, 16, COLS, M).transpose(0, 3, 1, 2)   # (b, j, kb, c)
    return np.ascontiguousarray(t.reshape(BPC, M, M))


def _pack_core(alpha_core, f_rhs):
    """Build the per-core input map (all fp32 numpy arrays)."""
    diag, KL, KB = _coeff_arrays(alpha_core)
    s = HINV2
    cD = _to_dev(diag * s)                               # [128, 4096]
    nKL = _to_dev(KL * (-s)).reshape(128, COLS, M)       # (P, c, j)
    nKB = _to_dev(KB * (-s)).reshape(128, COLS, M)

    # cLp[P, c, 0..256]: 0 at jj=0 (Dirichlet kill for the j-1 shift),
    # -s*KL[jj,k] at jj=1..255, 0 at jj=256 (K_right mask at j=255).
    cLp = np.zeros((128, COLS, M + 1), np.float32)
    cLp[:, :, 1:M] = nKL[:, :, 1:M]

    # cBp[P, 0..16, j]: c=0..15 the (already k-masked) bottom coefficients,
    # c=16 the next partition's c=0 column (static k-halo; 0 past k=255).
    cBp = np.zeros((128, COLS + 1, M), np.float32)
    cBp[:, :COLS, :] = nKB
    nKB4 = nKB.reshape(BPC, 16, COLS, M)
    cBp4 = cBp.reshape(BPC, 16, COLS + 1, M)
    cBp4[:, :-1, COLS, :] = nKB4[:, 1:, 0, :]

    fdev = _to_dev(np.broadcast_to(f_rhs, (BPC, M, M)).astype(np.float32))

    seg = np.zeros((128, BPC), np.float32)               # seg[q, b] = q//16 == b
    seg[np.arange(128), np.arange(128) // 16] = 1.0
    bc = np.ascontiguousarray(seg.T)                     # (8, 128)
    qi = np.arange(128)
    bc128 = (qi[:, None] // 16 == qi[None, :] // 16).astype(np.float32)
    sdn = np.eye(128, 128, 1, np.float32)                # out[i] = in[i-1]
    sup = np.eye(128, 128, -1, np.float32)               # out[i] = in[i+1]

    return {
        "f_in": fdev,
        "cD_in": cD,
        "cL_in": np.ascontiguousarray(cLp.reshape(128, COLS * (M + 1))),
        "cB_in": np.ascontiguousarray(cBp.reshape(128, (COLS + 1) * M)),
        "seg_in": seg,
        "bc_in": bc,
        "bc128_in": bc128,
        "sdn_in": sdn,
        "sup_in": sup,
    }


# --------------------------------------------------------------- bass kernel

def _build_nc_qrec(iters):
    """q-recurrence variant: q_{k+1} = A r_{k+1} + beta_k q_k.

    The stencil runs on r (available right after the r update), so the
    ||r||^2 / beta / p-update chain hides behind it. Validated in exp3.py:
    lands as close to the f64 trajectory as plain fp32 CG.

    Loop state: p, q (= A p), r (halo'd), x, gamvec ([128,1] per-problem
    gamma broadcast). Body:
        pAp = <p, q>; alpha = gamma/pAp
        x += alpha p ; r -= alpha q ; refresh r halos
        gamma' = ||r||^2 ; beta = gamma'/gamma
        w = A r  (overlaps beta chain and p update)
        p = r + beta p ; q = w + beta q
    """
    from contextlib import ExitStack
    import concourse.bass as bass
    import concourse.tile as tile
    from concourse import bacc, mybir

    f32 = mybir.dt.float32
    Alu = mybir.AluOpType
    Act = mybir.ActivationFunctionType

    nc = bacc.Bacc("TRN2", target_bir_lowering=False, debug=False)

    f_d = nc.dram_tensor("f_in", [128, F], f32, kind="ExternalInput").ap()
    cD_d = nc.dram_tensor("cD_in", [128, F], f32, kind="ExternalInput").ap()
    cL_d = nc.dram_tensor("cL_in", [128, COLS * (M + 1)], f32, kind="ExternalInput").ap()
    cB_d = nc.dram_tensor("cB_in", [128, (COLS + 1) * M], f32, kind="ExternalInput").ap()
    bc128_d = nc.dram_tensor("bc128_in", [128, 128], f32, kind="ExternalInput").ap()
    sdn_d = nc.dram_tensor("sdn_in", [128, 128], f32, kind="ExternalInput").ap()
    sup_d = nc.dram_tensor("sup_in", [128, 128], f32, kind="ExternalInput").ap()
    x_d = nc.dram_tensor("x_out", [128, F], f32, kind="ExternalOutput").ap()

    with tile.TileContext(nc) as tc, ExitStack() as ctx:
        sb = ctx.enter_context(tc.tile_pool(name="state", bufs=1))
        ps = ctx.enter_context(tc.tile_pool(name="psum", bufs=1, space="PSUM"))

        r = sb.tile([128, FH], f32)       # halo_lo | center | halo_hi
        p = sb.tile([128, F], f32)
        x = sb.tile([128, F], f32)
        q = sb.tile([128, F], f32)        # A @ p via recurrence
        t0 = sb.tile([128, F], f32)
        t1 = sb.tile([128, F], f32)
        t2 = sb.tile([128, F], f32)
        t3 = sb.tile([128, F], f32)
        cD = sb.tile([128, F], f32)
        cL = sb.tile([128, COLS * (M + 1)], f32)
        cB = sb.tile([128, (COLS + 1) * M], f32)
        bc128 = sb.tile([128, 128], f32)
        sdn = sb.tile([128, 128], f32)
        sup = sb.tile([128, 128], f32)

        pap_part = sb.tile([128, 1], f32)
        rr_part = sb.tile([128, 1], f32)
        gamvec = sb.tile([128, 1], f32)   # per-problem gamma, broadcast
        recg = sb.tile([128, 1], f32)     # 1/gamma_old
        recp = sb.tile([128, 1], f32)     # 1/pAp
        avec = sb.tile([128, 1], f32)
        bvec = sb.tile([128, 1], f32)

        pap_ps = ps.tile([128, 1], f32)
        gam_ps = ps.tile([128, 1], f32)
        hlo_ps = ps.tile([128, M], f32)
        hhi_ps = ps.tile([128, M], f32)

        def v3(ap2d):
            return ap2d.rearrange("p (c j) -> p c j", c=COLS, j=M)

        r_c2 = r[:, M:M + F]
        r_c3 = v3(r_c2)
        r_jm1 = v3(r[:, M - 1:M - 1 + F])
        r_jp1 = v3(r[:, M + 1:M + 1 + F])
        r_km1 = v3(r[:, 0:F])
        r_kp1 = v3(r[:, 2 * M:2 * M + F])
        cL3 = cL[:].rearrange("p (c j) -> p c j", c=COLS, j=M + 1)
        cLl = cL3[:, :, 0:M]
        cLr = cL3[:, :, 1:M + 1]
        cB3 = cB[:].rearrange("p (c j) -> p c j", c=COLS + 1, j=M)
        cBb = cB3[:, 0:COLS, :]
        cBt = cB3[:, 1:COLS + 1, :]
        cD3 = v3(cD[:])

        nc.sync.dma_start(cD[:], cD_d)
        nc.sync.dma_start(cL[:], cL_d)
        nc.sync.dma_start(cB[:], cB_d)
        nc.sync.dma_start(bc128[:], bc128_d)
        nc.sync.dma_start(sdn[:], sdn_d)
        nc.sync.dma_start(sup[:], sup_d)
        nc.sync.dma_start(r_c2, f_d)
        nc.sync.dma_start(p[:], f_d)

        def halo_update():
            nc.tensor.matmul(hlo_ps[:], sdn[:], r[:, F:F + M])
            nc.tensor.matmul(hhi_ps[:], sup[:], r[:, M:2 * M])
            nc.scalar.copy(r[:, 0:M], hlo_ps[:])
            nc.scalar.copy(r[:, F + M:F + 2 * M], hhi_ps[:])

        def stencil_w():
            """t0 = A @ r (j-terms on DVE, k-products on GpSimd)."""
            nc.gpsimd.tensor_mul(v3(t2[:]), cBb, r_km1)
            nc.gpsimd.tensor_mul(v3(t3[:]), cBt, r_kp1)
            nc.vector.tensor_mul(v3(t0[:]), cD3, r_c3)
            nc.vector.tensor_mul(v3(t1[:]), cLl, r_jm1)
            nc.vector.tensor_add(t0[:], t0[:], t1[:])
            nc.vector.tensor_mul(v3(t1[:]), cLr, r_jp1)
            nc.vector.tensor_add(t0[:], t0[:], t1[:])
            nc.vector.tensor_add(t0[:], t0[:], t2[:])
            nc.vector.tensor_add(t0[:], t0[:], t3[:])

        # ---- init: x=0, r=p=f, q = A p, gamma0
        nc.vector.memset(x[:], 0.0)
        halo_update()
        nc.scalar.activation(t1[:], r_c2, Act.Square, accum_out=rr_part[:])
        nc.tensor.matmul(gam_ps[:], bc128[:], rr_part[:])
        nc.scalar.copy(gamvec[:], gam_ps[:])
        stencil_w()
        nc.vector.tensor_copy(q[:], t0[:])

        # ---- 300 CG iterations
        with tc.For_i(0, iters) as _i:
            nc.vector.reciprocal(recg[:], gamvec[:])

            # pAp and alpha
            nc.vector.tensor_mul(t3[:], p[:], q[:])
            nc.scalar.activation(t3[:], t3[:], Act.Copy, accum_out=pap_part[:])
            nc.tensor.matmul(pap_ps[:], bc128[:], pap_part[:])
            nc.vector.reciprocal(recp[:], pap_ps[:])
            nc.vector.tensor_mul(avec[:], gamvec[:], recp[:])

            # x += alpha*p (ACT+GpSimd, off critical) ; r -= alpha*q (DVE)
            nc.scalar.activation(t2[:], p[:], Act.Copy, scale=avec[:])
            nc.gpsimd.tensor_add(x[:], x[:], t2[:])
            nc.vector.tensor_scalar_mul(t1[:], q[:], avec[:])
            nc.vector.tensor_sub(r_c2, r_c2, t1[:])
            halo_update()

            # gamma' and beta (hidden under the stencil)
            nc.scalar.activation(t1[:], r_c2, Act.Square, accum_out=rr_part[:])
            nc.tensor.matmul(gam_ps[:], bc128[:], rr_part[:])
            nc.vector.tensor_mul(bvec[:], gam_ps[:], recg[:])
            nc.scalar.copy(gamvec[:], gam_ps[:])

            # w = A r
            stencil_w()

            # p = r + beta*p (GpSimd) ; q = w + beta*q (DVE)
            nc.gpsimd.tensor_scalar_mul(t2[:], p[:], bvec[:])
            nc.gpsimd.tensor_add(p[:], r_c2, t2[:])
            nc.vector.tensor_scalar_mul(t1[:], q[:], bvec[:])
            nc.vector.tensor_add(q[:], t0[:], t1[:])

        nc.sync.dma_start(x_d, x[:])

    nc.compile()
    return nc


def _build_nc(iters):
    from contextlib import ExitStack
    import concourse.bass as bass
    import concourse.tile as tile
    from concourse import bacc, mybir

    f32 = mybir.dt.float32
    Alu = mybir.AluOpType
    Act = mybir.ActivationFunctionType

    nc = bacc.Bacc("TRN2", target_bir_lowering=False, debug=False)

    f_d = nc.dram_tensor("f_in", [128, F], f32, kind="ExternalInput").ap()
    cD_d = nc.dram_tensor("cD_in", [128, F], f32, kind="ExternalInput").ap()
    cL_d = nc.dram_tensor("cL_in", [128, COLS * (M + 1)], f32, kind="ExternalInput").ap()
    cB_d = nc.dram_tensor("cB_in", [128, (COLS + 1) * M], f32, kind="ExternalInput").ap()
    bc128_d = nc.dram_tensor("bc128_in", [128, 128], f32, kind="ExternalInput").ap()
    sdn_d = nc.dram_tensor("sdn_in", [128, 128], f32, kind="ExternalInput").ap()
    sup_d = nc.dram_tensor("sup_in", [128, 128], f32, kind="ExternalInput").ap()
    x_d = nc.dram_tensor("x_out", [128, F], f32, kind="ExternalOutput").ap()

    with tile.TileContext(nc) as tc, ExitStack() as ctx:
        sb = ctx.enter_context(tc.tile_pool(name="state", bufs=1))
        ps = ctx.enter_context(tc.tile_pool(name="psum", bufs=1, space="PSUM"))

        p = sb.tile([128, FH], f32)       # halo_lo | center | halo_hi
        r = sb.tile([128, F], f32)
        x = sb.tile([128, F], f32)
        q = sb.tile([128, F], f32)        # A @ p
        t0 = sb.tile([128, F], f32)       # DVE stencil accumulator
        t1 = sb.tile([128, F], f32)       # DVE-only scratch (products, axpy terms)
        t2 = sb.tile([128, F], f32)       # GpSimd m3 product / ACT rr junk
        t3 = sb.tile([128, F], f32)       # GpSimd m4 product / pAp product / x term
        t4 = sb.tile([128, F], f32)       # GpSimd m1 product (dedicated)
        cD = sb.tile([128, F], f32)
        cL = sb.tile([128, COLS * (M + 1)], f32)
        cB = sb.tile([128, (COLS + 1) * M], f32)
        bc128 = sb.tile([128, 128], f32)
        sdn = sb.tile([128, 128], f32)
        sup = sb.tile([128, 128], f32)

        pap_part = sb.tile([128, 1], f32)
        rr_part = sb.tile([128, 1], f32)
        gamvec = sb.tile([128, 1], f32)   # per-problem gamma, broadcast
        recg = sb.tile([128, 1], f32)
        recp = sb.tile([128, 1], f32)
        avec = sb.tile([128, 1], f32)
        aneg = sb.tile([128, 1], f32)
        bvec = sb.tile([128, 1], f32)

        pap_ps = ps.tile([128, 1], f32)
        gam_ps = ps.tile([128, 1], f32)
        hlo_ps = ps.tile([128, M], f32)
        hhi_ps = ps.tile([128, M], f32)

        # 3D views [128, 16, 256] over the stencil operands
        def v3(ap2d):
            return ap2d.rearrange("p (c j) -> p c j", c=COLS, j=M)

        p_c2 = p[:, M:M + F]
        p_c3 = v3(p_c2)
        p_jm1 = v3(p[:, M - 1:M - 1 + F])
        p_jp1 = v3(p[:, M + 1:M + 1 + F])
        p_km1 = v3(p[:, 0:F])
        p_kp1 = v3(p[:, 2 * M:2 * M + F])
        cL3 = cL[:].rearrange("p (c j) -> p c j", c=COLS, j=M + 1)
        cLl = cL3[:, :, 0:M]        # multiplies p_jm1
        cLr = cL3[:, :, 1:M + 1]    # multiplies p_jp1 (= K_right view)
        cB3 = cB[:].rearrange("p (c j) -> p c j", c=COLS + 1, j=M)
        cBb = cB3[:, 0:COLS, :]     # multiplies p_km1
        cBt = cB3[:, 1:COLS + 1, :] # multiplies p_kp1 (= K_top view)
        cD3 = v3(cD[:])

        # ---- load inputs
        nc.sync.dma_start(cD[:], cD_d)
        nc.sync.dma_start(cL[:], cL_d)
        nc.sync.dma_start(cB[:], cB_d)
        nc.sync.dma_start(bc128[:], bc128_d)
        nc.sync.dma_start(sdn[:], sdn_d)
        nc.sync.dma_start(sup[:], sup_d)
        nc.sync.dma_start(r[:], f_d)
        nc.sync.dma_start(p_c2, f_d)

        def halo_update():
            # halo_lo[P] = center_last_col[P-1]; halo_hi[P] = center_first_col[P+1]
            nc.tensor.matmul(hlo_ps[:], sdn[:], p[:, F:F + M])
            nc.tensor.matmul(hhi_ps[:], sup[:], p[:, M:2 * M])
            nc.scalar.copy(p[:, 0:M], hlo_ps[:])
            nc.scalar.copy(p[:, F + M:F + 2 * M], hhi_ps[:])

        # ---- init: x=0, gamma0 = per-problem ||f||^2, p halos
        nc.vector.memset(x[:], 0.0)
        halo_update()
        nc.scalar.activation(t1[:], r[:], Act.Square, accum_out=rr_part[:])
        nc.tensor.matmul(gam_ps[:], bc128[:], rr_part[:])
        nc.scalar.copy(gamvec[:], gam_ps[:])

        # ---- 300 CG iterations
        loop_mode = os.environ.get("KERNEL_LOOP", "unroll4")

        def body(_i):
            # 1/gamma_old for beta, overlappable with the stencil
            nc.vector.reciprocal(recg[:], gamvec[:])

            # q = A @ p  (GpSimd: k-shift products; DVE: the rest)
            nc.gpsimd.tensor_mul(v3(t2[:]), cBb, p_km1)
            nc.gpsimd.tensor_mul(v3(t3[:]), cBt, p_kp1)
            nc.vector.tensor_mul(v3(t0[:]), cD3, p_c3)
            nc.vector.tensor_mul(v3(t1[:]), cLl, p_jm1)
            nc.vector.tensor_add(t0[:], t0[:], t1[:])
            nc.vector.tensor_mul(v3(t1[:]), cLr, p_jp1)
            nc.vector.tensor_add(t0[:], t0[:], t1[:])
            nc.vector.tensor_add(t0[:], t0[:], t2[:])
            nc.vector.tensor_add(q[:], t0[:], t3[:])

            # pAp = sum(p*q) fused in one DVE pass; alpha = gamma/pAp
            nc.vector.scalar_tensor_tensor(
                t3[:], p_c2, 1.0, q[:], Alu.mult, Alu.mult,
                accum_out=pap_part[:])
            nc.tensor.matmul(pap_ps[:], bc128[:], pap_part[:])
            nc.vector.reciprocal(recp[:], pap_ps[:])
            nc.vector.tensor_mul(avec[:], gamvec[:], recp[:])
            nc.vector.tensor_scalar_mul(aneg[:], avec[:], -1.0)

            # r = (q * -alpha) + r, one pass; x += alpha*p off-critical
            nc.vector.scalar_tensor_tensor(
                r[:], q[:], aneg[:], r[:], Alu.mult, Alu.add)
            nc.scalar.activation(t3[:], p_c2, Act.Copy, scale=avec[:])
            nc.gpsimd.tensor_add(x[:], x[:], t3[:])

            # gamma' = sum(r*r) fused on DVE (no engine hop); beta
            nc.vector.scalar_tensor_tensor(
                t2[:], r[:], 1.0, r[:], Alu.mult, Alu.mult,
                accum_out=rr_part[:])
            nc.tensor.matmul(gam_ps[:], bc128[:], rr_part[:])
            nc.vector.tensor_mul(bvec[:], gam_ps[:], recg[:])
            nc.scalar.copy(gamvec[:], gam_ps[:])

            # p = (p * beta) + r in one pass, then refresh halos
            nc.vector.scalar_tensor_tensor(
                p_c2, p_c2, bvec[:], r[:], Alu.mult, Alu.add)
            halo_update()

        if loop_mode == "plain":
            with tc.For_i(0, iters) as _i:
                body(_i)
        elif loop_mode == "stag":
            with tc.For_i(0, iters, staggered_reset=True) as _i:
                body(_i)
        elif loop_mode.startswith("unroll"):
            tc.For_i_unrolled(0, iters, 1, body, max_unroll=int(loop_mode[6:]))
        else:
            raise ValueError(loop_mode)

        nc.sync.dma_start(x_d, x[:])

    nc.compile()
    return nc


VARIANT = os.environ.get("KERNEL_VARIANT", "std")


def _get_nc(iters, variant=None):
    variant = variant or VARIANT
    key = ("nc", iters, variant, os.environ.get("KERNEL_LOOP", "unroll4"))
    if key not in _CACHE:
        builder = {"std": _build_nc, "qrec": _build_nc_qrec}[variant]
        _CACHE[key] = builder(iters)
    return _CACHE[key]


def _expected_inputs(nc):
    import concourse.mybir as mybir
    part = nc.partition_id_tensor.name if nc.partition_id_tensor else None
    names = set()
    for alloc in nc.m.functions[0].allocations:
        if isinstance(alloc, mybir.MemoryLocationSet) and alloc.kind == "ExternalInput":
            nm = alloc.memorylocations[0].name
            if nm != part:
                names.add(nm)
    return names


# ------------------------------------------------------------------- runner

def _make_runner(iters, variant=None):
    """Build the 8-core sharded jit once; returns run(in_maps) -> [x_out]*8."""
    import jax
    from jax.sharding import Mesh, PartitionSpec
    from jax.experimental.shard_map import shard_map
    from concourse import bass2jax, mybir

    nc = _get_nc(iters, variant)
    bass2jax.install_neuronx_cc_hook()
    partition_name = nc.partition_id_tensor.name if nc.partition_id_tensor else None
    in_names, out_names, out_avals, zero_outs = [], [], [], []
    for alloc in nc.m.functions[0].allocations:
        if not isinstance(alloc, mybir.MemoryLocationSet):
            continue
        name = alloc.memorylocations[0].name
        if alloc.kind == "ExternalInput":
            if name != partition_name:
                in_names.append(name)
        elif alloc.kind == "ExternalOutput":
            out_names.append(name)
            shape = tuple(alloc.tensor_shape)
            dtype = mybir.dt.np(alloc.dtype)
            out_avals.append(jax.core.ShapedArray(shape, dtype))
            zero_outs.append(np.zeros(shape, dtype))
    n_params = len(in_names)
    all_in = in_names + out_names + ([partition_name] if partition_name else [])

    def _body(*args):
        ops = list(args)
        if partition_name:
            ops.append(bass2jax.partition_id_tensor())
        return tuple(bass2jax._bass_exec_p.bind(
            *ops, out_avals=tuple(out_avals), in_names=tuple(all_in),
            out_names=tuple(out_names), lowering_input_output_aliases=(),
            sim_require_finite=True, sim_require_nnan=True, nc=nc))

    mesh = Mesh(np.asarray(jax.devices()[:NCORES]), ("core",))
    jf = jax.jit(
        shard_map(_body, mesh=mesh,
                  in_specs=(PartitionSpec("core"),) * (n_params + len(out_names)),
                  out_specs=(PartitionSpec("core"),) * len(out_names),
                  check_rep=False),
        donate_argnums=tuple(range(n_params, n_params + len(out_names))),
        keep_unused=True)

    def prepare(in_maps):
        import jax
        concat_in = [np.concatenate([m[nm] for m in in_maps], axis=0)
                     for nm in in_names]
        dev_in = [jax.device_put(a) for a in concat_in]
        jax.block_until_ready(dev_in)
        return dev_in

    def run_dev(dev_in, fetch=True):
        import jax
        zeros = [np.zeros((NCORES * z.shape[0], *z.shape[1:]), z.dtype)
                 for z in zero_outs]
        outs = jf(*dev_in, *zeros)
        if not fetch:
            jax.block_until_ready(outs)
            return None
        xo = np.asarray(outs[out_names.index("x_out")])
        per_core_rows = xo.shape[0] // NCORES
        return [xo[c * per_core_rows:(c + 1) * per_core_rows] for c in range(NCORES)]

    def run(in_maps):
        return run_dev(prepare(in_maps))

    run.prepare = prepare
    run.run_dev = run_dev
    return run


def _get_runner(iters, variant=None):
    variant = variant or VARIANT
    key = ("runner", iters, variant, os.environ.get("KERNEL_LOOP", "unroll4"))
    if key not in _CACHE:
        _CACHE[key] = _make_runner(iters, variant)
    return _CACHE[key]


def _run(in_maps, iters, variant=None):
    return _get_runner(iters, variant)(in_maps)


def kernel(alpha, f_rhs):
    alpha = np.asarray(alpha, np.float32)
    f_rhs = np.asarray(f_rhs, np.float32)
    in_maps = [_pack_core(alpha[c * BPC:(c + 1) * BPC], f_rhs)
               for c in range(NCORES)]
    try:
        outs = _run(in_maps, ITERS)
    except Exception:
        # a crashed prior session can leave a core wedged; one retry clears it
        outs = _run(in_maps, ITERS)
    return np.concatenate([_from_dev(o) for o in outs], axis=0)
